# revision 2
# baseline (speedup 1.0000x reference)
"""Trainium2 Bass kernel: BertUnpadSelfAttention (B=8, S=1024, H=12, D=64).

Strategy
--------
Data-parallel over batch: core b handles batch b (all 12 heads).

Host prep (per call):
  * scatter unpadded hidden rows into dense [B*S, 768] (+ valid-row flag), like
    the reference's pad_input
  * fold the 1/sqrt(D) score scale into the W/bias q-columns
  * transpose: hT_aug = [hidden_padded | valid]^T  -> [769, 1024] fp16 per core
  * Eb = exp(bias) * 2^-4 transposed to [H, k, q] fp16 per core
    (so that softmax(s+bias) = (exp(s)*Eb) / sum(exp(s)*Eb); the 2^-4 scale
    cancels in the normalization and keeps products in fp16 range)

Device (per core), all matmuls fp16 -> fp32 PSUM:
  * qkvT = W_aug^T @ hT_aug: q^T,k^T stored [d-pairs on partitions, t] so that
    per-head scores^T need no transposes anywhere; v stored [t, h, d|1] with a
    ones column appended (gives the softmax denominator via the same matmul)
  * per head: scores^T tile = k^T.T @ q^T (PSUM), exp on ScalarE -> fp16,
    multiply by Eb tile on VectorE -> p^T fp16, p@[v|1] accumulated over k
    chunks into PSUM [q,65]; normalize rows by 1/sum and stage to SBUF
  * single output DMA [1024, 768] fp32

Host post: gather rows by `indices` -> (nnz, 768).
"""

import numpy as np

B, S, H, D = 8, 1024, 12, 64
HID = H * D            # 768
BS = B * S             # 8192
NC = 8                 # cores
EB_SCALE = 0.0625      # folded into exp(bias); cancels in softmax

_CACHE = {}


def _build_nc():
    import concourse.mybir as mybir
    import concourse.tile as tile
    from concourse import bacc

    f16 = mybir.dt.float16
    f32 = mybir.dt.float32
    Exp = mybir.ActivationFunctionType.Exp

    nc = bacc.Bacc("TRN2", debug=False, num_devices=NC)
    hT = nc.dram_tensor("hT", [769, S], f16, kind="ExternalInput").ap()
    W = nc.dram_tensor("W", [769, 3 * HID], f16, kind="ExternalInput").ap()
    EbT = nc.dram_tensor("EbT", [H, S, S], f16, kind="ExternalInput").ap()
    out = nc.dram_tensor("out", [S, HID], f32, kind="ExternalOutput").ap()

    with tile.TileContext(nc) as tc:
        with (
            tc.tile_pool(name="per", bufs=1) as per,
            tc.tile_pool(name="ebp", bufs=2) as ebp,
            tc.tile_pool(name="st", bufs=3) as st,
            tc.tile_pool(name="psm", bufs=3, space="PSUM") as psm,
            tc.tile_pool(name="pat", bufs=5, space="PSUM") as pat,
        ):
            # ---- persistent loads -------------------------------------------
            hT_sb = per.tile([128, 6, S], f16)
            for ic in range(6):
                nc.sync.dma_start(hT_sb[:, ic], hT[ic * 128:(ic + 1) * 128, :])
            hT_last = per.tile([1, S], f16)
            nc.sync.dma_start(hT_last, hT[768:769, :])
            W_sb = per.tile([128, 6, 3 * HID], f16)
            for ic in range(6):
                nc.sync.dma_start(W_sb[:, ic], W[ic * 128:(ic + 1) * 128, :])
            W_last = per.tile([1, 3 * HID], f16)
            nc.sync.dma_start(W_last, W[768:769, :])

            # q^T/k^T: [128 = head-pair d dims, pair, t]; head 2p+half lives on
            # partitions half*64..half*64+63 of pair p (feeds row-group packing)
            qT_sb = per.tile([128, 6, S], f16)
            kT_sb = per.tile([128, 6, S], f16)
            # v with ones column: [t_in_chunk, t_chunk, head, d+1]
            vv = per.tile([128, 8, H, D + 1], f16)
            out_sb = per.tile([128, 8, HID], f32)
            nc.vector.memset(vv, 1.0)

            # ---- q/k projection: qkvT[c, t] for c-chunks 0..11 (q then k) ---
            for ci in range(12):
                dest = qT_sb if ci < 6 else kT_sb
                pair = ci % 6
                for t2 in range(2):
                    ps = psm.tile([128, 512], f32, tag="mm", name="ps_qk")
                    for ic in range(6):
                        nc.tensor.matmul(
                            ps,
                            W_sb[:, ic, ci * 128:(ci + 1) * 128],
                            hT_sb[:, ic, t2 * 512:(t2 + 1) * 512],
                            start=(ic == 0), stop=False,
                        )
                    nc.tensor.matmul(
                        ps,
                        W_last[:, ci * 128:(ci + 1) * 128],
                        hT_last[:, t2 * 512:(t2 + 1) * 512],
                        start=False, stop=True,
                    )
                    nc.scalar.copy(dest[:, pair, t2 * 512:(t2 + 1) * 512], ps)

            # ---- v projection: v[t, c] -------------------------------------
            for t8 in range(8):
                for (n0, nw) in ((0, 512), (512, 256)):
                    ps = psm.tile([128, nw], f32, tag="mm", name="ps_v")
                    for ic in range(6):
                        nc.tensor.matmul(
                            ps,
                            hT_sb[:, ic, t8 * 128:(t8 + 1) * 128],
                            W_sb[:, ic, 2 * HID + n0:2 * HID + n0 + nw],
                            start=(ic == 0), stop=False,
                        )
                    nc.tensor.matmul(
                        ps,
                        hT_last[:, t8 * 128:(t8 + 1) * 128],
                        W_last[:, 2 * HID + n0:2 * HID + n0 + nw],
                        start=False, stop=True,
                    )
                    h0, nh = n0 // 64, nw // 64
                    nc.vector.tensor_copy(
                        vv[:, t8, h0:h0 + nh, 0:D],
                        ps.rearrange("p (h d) -> p h d", d=D),
                    )

            # ---- attention -------------------------------------------------
            for h in range(H):
                pair, half = h // 2, h % 2
                p0 = half * 64
                eb = ebp.tile([128, 8, S], f16, tag="eb", name="eb")
                nc.sync.dma_start(eb, EbT[h].rearrange("(kc p) q -> p kc q", p=128))
                for qc in range(2):
                    att = [
                        pat.tile([128, D + 1], f32, tag="at", name="att")
                        for _ in range(4)
                    ]
                    for kc in range(8):
                        sps = psm.tile([128, 512], f32, tag="mm", name="sps")
                        nc.tensor.matmul(
                            sps,
                            kT_sb[p0:p0 + 64, pair, kc * 128:(kc + 1) * 128],
                            qT_sb[p0:p0 + 64, pair, qc * 512:(qc + 1) * 512],
                            start=True, stop=True,
                        )
                        es = st.tile([128, 512], f16, tag="es", name="es")
                        nc.scalar.activation(es, sps, Exp)
                        pt = st.tile([128, 512], f16, tag="pt", name="pt")
                        nc.vector.tensor_mul(pt, es, eb[:, kc, qc * 512:(qc + 1) * 512])
                        for j in range(4):
                            nc.tensor.matmul(
                                att[j],
                                pt[:, j * 128:(j + 1) * 128],
                                vv[:, kc, h, :],
                                start=(kc == 0), stop=(kc == 7),
                            )
                    for j in range(4):
                        rec = st.tile([128, 1], f32, tag="rec", bufs=6, name="rec")
                        nc.vector.reciprocal(rec, att[j][:, D:D + 1])
                        dst = out_sb[:, qc * 4 + j, h * D:(h + 1) * D]
                        if j % 2 == 0:
                            nc.vector.tensor_scalar_mul(dst, att[j][:, 0:D], rec)
                        else:
                            nc.scalar.mul(dst, att[j][:, 0:D], rec)

            # ---- store -----------------------------------------------------
            for t8 in range(8):
                nc.sync.dma_start(out[t8 * 128:(t8 + 1) * 128, :], out_sb[:, t8])
    nc.compile()
    return nc


def _get_nc():
    if "nc" not in _CACHE:
        _CACHE["nc"] = _build_nc()
    return _CACHE["nc"]


def prepare_in_maps(inputs):
    """Host-side shard/prep: returns (in_maps list for 8 cores, indices)."""
    hidden = np.asarray(inputs["hidden_states"], np.float32)
    W = np.array(np.asarray(inputs["Wqkv_w"], np.float32))
    b = np.array(np.asarray(inputs["Wqkv_b"], np.float32))
    bias = np.asarray(inputs["bias"], np.float32)
    indices = np.asarray(inputs["indices"], np.int32)

    scale = 1.0 / np.sqrt(np.float32(D))
    Ws = W.copy()
    Ws[:, :HID] *= scale
    bs = b.copy()
    bs[:HID] *= scale
    W_aug = np.concatenate([Ws, bs[None, :]], axis=0).astype(np.float16)

    hp = np.zeros((BS, HID), np.float32)
    hp[indices] = hidden
    valid = np.zeros((1, BS), np.float32)
    valid[0, indices] = 1.0

    def prep_core(c):
        hTa = np.concatenate(
            [hp[c * S:(c + 1) * S].T, valid[:, c * S:(c + 1) * S]], axis=0
        ).astype(np.float16)
        ebt = np.empty((H, S, S), np.float16)
        for h in range(H):
            ebt[h] = (np.exp(bias[c, h]) * EB_SCALE).T.astype(np.float16)
        return {"hT": hTa, "W": W_aug, "EbT": ebt}

    from concurrent.futures import ThreadPoolExecutor
    with ThreadPoolExecutor(max_workers=8) as ex:
        in_maps = list(ex.map(prep_core, range(NC)))
    return in_maps, indices


def _run_spmd(in_maps, trace=False):
    from concourse.bass_utils import run_bass_kernel_spmd
    return run_bass_kernel_spmd(
        _get_nc(), in_maps, core_ids=list(range(NC)), trace=trace
    )


def kernel(**inputs):
    in_maps, indices = prepare_in_maps(inputs)
    res = _run_spmd(in_maps, trace=False)
    full = np.empty((BS, HID), np.float32)
    for c in range(NC):
        full[c * S:(c + 1) * S] = res.results[c]["out"]
    return full[indices]
